# revision 18
# baseline (speedup 1.0000x reference)
"""CustomGAT (gnn_message_passing) Trainium2 kernel — 8-core SPMD, v2.

Strategy (dst-partitioned edge parallelism, zero output collectives):
  * Host: add self-loops. Table rows use IDENTITY src mapping (node id ->
    table row), so the dma_gather range split (ucode offset limit) is fixed
    up front and the dst-block assignment can be balanced against per-range
    chunk maxima directly (multi-range greedy LPT -> 19 padded chunks per
    block instead of 22). Fold attn_l/attn_r into the projection weights.
  * Device phase A (replicated on each core): projection table
    Tab[slots, 384] bf16 rows = [xp x256 | al x8 | ar x8 | pad] written to
    HBM scratch. bf16 x input halves the read traffic vs f32; PSUM->SBUF
    converts alternate between the vector and activation engines.
  * Device phase A'': tiny second matmul pass over host-permuted x (own
    nodes in output order) emits the per-core dst-side ar table directly —
    no indirect DMA.
  * Device phase B (per supergroup of SG_B dst-blocks): consolidated
    dma_gather calls (<=1024 indices each — the deployed ucode crashes
    above that). Per block: one batched is_equal builds all one-hot
    scatter matrices, one batched add/lrelu/exp chain produces alpha
    (activation engine), one batched multiply per range forms the
    messages, then one-hot matmuls accumulate in PSUM (alpha ride-along
    column = softmax denominator) and a per-head normalize writes out.
  * Host: concatenate per-core output shards, inverse-permute slots.

Env knobs (defaults are the validated config): K_NQ (SWDGE queues),
K_GMAX (chunks per gather call), K_SEMPATCH (queue-aware sem rotation).
"""

import math

import numpy as np

# ---------------------------------------------------------------- constants
H = 8
C = 32
HC = H * C  # 256
IN = 256
ROW = 384  # bf16 slots: [xp 0:256 | al f32 256:272 | ar f32 272:288 | pad]
PSROW = HC + 2 * H  # 272 fp32 projection row [xp | al | ar]
P = 128
# dma_gather runtime offset (idx * row_bytes) tops out near 2^24 bytes;
# with 768B rows that caps idx at 21845. Split the table into ranges.
RNG = 21504
SG_B = 2  # dst-blocks per gather supergroup
import os as _os
NQ = int(_os.environ.get('K_NQ', '4'))  # SWDGE queues
GMAX = int(_os.environ.get('K_GMAX', '8'))  # max chunks per gather call
SEMPATCH = int(_os.environ.get('K_SEMPATCH', '0'))


# ---------------------------------------------------------------- tile patch
def _install_tile_patch():
    """The axon-path walrus rejects >2 sync waits on one instruction; split
    the TileContext tail-drain waits into one carrier drain per proc."""
    import concourse.tile as tile
    from concourse.vector_clock import ScopedClock, VectorClock

    if getattr(tile.TileContext, "_drain_patch_installed", False):
        return

    def _drain_and_barrier(self, tick_clock, wait_clock):
        gc = tick_clock.global_clock
        n = len(gc)
        for p in range(n):
            if gc[p] == 0:
                continue
            req = VectorClock([gc[q] if q == p else 0 for q in range(n)])
            d = self.nc.sync.drain()
            wait_clock.add_sem_waits(d.ins, ScopedClock({None: req}))
        self.nc.all_engine_barrier()
        assert self.sems is not None
        popped = self.nc._tile_sem_poison_stack.pop()
        assert popped is self._sem_poison
        self.nc.clear_and_free_semaphores(list(self.sems.allocated().values()))
        self.nc.all_engine_barrier()

    tile.TileContext._drain_and_barrier = _drain_and_barrier
    tile.TileContext._drain_patch_installed = True

    # Queue-aware SWDGE semaphore assignment: each dma_gather queue gets its
    # own pair of DMASW sems (the sim/hw lock each sem to one queue; the
    # stock round-robin ignores queue_num and mixes them).
    import concourse.tile_sem_assignment as tsa
    from concourse import mybir

    if not SEMPATCH:
        return
    orig_assign_tick = tsa.TileClockTick._assign_tick

    def _assign_tick_qaware(self, inst):
        if isinstance(inst, mybir.InstDMAGatherAnt):
            q = inst.queue_num
            tog = getattr(self, "_q_toggle", None)
            if tog is None:
                tog = self._q_toggle = {}
            t = tog.get(q, 0)
            tog[q] = t ^ 1
            saved = self.next_sw_dma_idx
            self.next_sw_dma_idx = (2 * q + t) % self.swdge_sem_count
            try:
                return orig_assign_tick(self, inst)
            finally:
                self.next_sw_dma_idx = saved
        return orig_assign_tick(self, inst)

    tsa.TileClockTick._assign_tick = _assign_tick_qaware


# ---------------------------------------------------------------- host prep
def _idx16(vals):
    """Encode row indices for dma_gather: vals [L] -> [128, L//16] int16,
    index k at [k%16 (+16*rep), k//16]. Concatenation-safe along columns."""
    enc = np.asarray(vals, dtype=np.int64).astype(np.int16)
    a = enc.reshape(-1, 16).T  # [16, L//16]
    return np.tile(a, (8, 1))  # replicate for the 8 Q7 cores


def _balance_blocks(dst, deg_rng, nblocks, n_ranges):
    """Greedy multi-range LPT: assign nodes to blocks (cap 128) minimizing
    the per-range count maxima (which set the padded chunk counts)."""
    N = deg_rng.shape[0]
    deg = deg_rng.sum(axis=1)
    order = np.argsort(-deg, kind="stable")
    counts = np.zeros((nblocks, n_ranges), dtype=np.int64)
    ncnt = np.zeros(nblocks, dtype=np.int64)
    mu = np.maximum(deg_rng.sum(axis=0) / nblocks, 1.0)  # per-range mean
    blk_of = np.empty(N, dtype=np.int64)
    slot_of = np.empty(N, dtype=np.int64)
    # penalty = max_r (counts + d) / mu ; vectorized over blocks per node
    for n in order:
        d = deg_rng[n]
        pen = ((counts + d[None, :]) / mu[None, :]).max(axis=1)
        pen[ncnt >= P] = np.inf
        b = int(np.argmin(pen))
        blk_of[n] = b
        slot_of[n] = ncnt[b]
        ncnt[b] += 1
        counts[b] += d
    return blk_of, slot_of, counts


def _preprocess(x, edge_index, W, attn_l, attn_r, n_cores):
    N = x.shape[0]

    src = np.concatenate([np.asarray(edge_index[0]), np.arange(N, dtype=np.int64)])
    dst = np.concatenate([np.asarray(edge_index[1]), np.arange(N, dtype=np.int64)])
    Etot = src.shape[0]

    bpc = math.ceil(N / (n_cores * P))  # blocks per core
    nblocks = n_cores * bpc
    slots = nblocks * P  # also number of table rows (identity + pad)

    # table row of node n is n itself -> edge range fixed before balancing
    nranges = max(1, math.ceil(slots / RNG))
    rid = src // RNG  # range of each edge (by src row)

    # per-node per-range in-edge counts
    deg_rng = np.zeros((N, nranges), dtype=np.int64)
    np.add.at(deg_rng, (dst, rid), 1)

    blk_of, slot_of, _ = _balance_blocks(dst, deg_rng, nblocks, nranges)
    out_row = blk_of * P + slot_of  # output shard row of node n

    eb = blk_of[dst]
    dloc = slot_of[dst]

    # per-(block, range) chunk counts
    nch = []
    for r in range(nranges):
        cnt_r = np.bincount(eb[rid == r], minlength=nblocks)
        nch.append(max(1, math.ceil((cnt_r.max() + 1) / P)))
    nchunk = sum(nch)
    cap = nchunk * P
    cbase = np.concatenate([[0], np.cumsum(nch)])  # chunk base per range

    gidx = np.zeros((nblocks, cap), dtype=np.int64)  # range-local src rows
    aidx = np.zeros((nblocks, cap), dtype=np.int64)  # block-local dst rows
    dlocp = np.full((nblocks, cap), 200.0, dtype=np.float32)
    for r in range(nranges):
        sel = rid == r
        e_r = np.where(sel)[0]
        order_r = np.argsort(eb[e_r], kind="stable")
        e_r = e_r[order_r]
        cnt_r = np.bincount(eb[sel], minlength=nblocks)
        s_r = np.concatenate([[0], np.cumsum(cnt_r)])
        pos_r = cbase[r] * P + np.arange(e_r.shape[0]) - s_r[eb[e_r]]
        gidx[eb[e_r], pos_r] = src[e_r] - r * RNG  # pads stay 0 (row 0)
        aidx[eb[e_r], pos_r] = dloc[e_r]
        dlocp[eb[e_r], pos_r] = dloc[e_r].astype(np.float32)

    # block-local tabAR rows: (b % bpc) * P + dloc
    aidx += (np.arange(nblocks)[:, None] % bpc) * P

    # per-core encoded index arrays, block-major (supergroup calls DMA a
    # contiguous span of blocks; _idx16 is concatenation-safe)
    xpi = {}
    for r in range(nranges):
        arr = np.empty((n_cores, bpc, P, nch[r] * 8), dtype=np.int16)
        for c in range(n_cores):
            for b in range(bpc):
                g = c * bpc + b
                arr[c, b] = _idx16(gidx[g, cbase[r] * P : cbase[r + 1] * P])
        xpi[r] = arr
    ari = np.empty((n_cores, bpc, P, nchunk * 8), dtype=np.int16)
    for c in range(n_cores):
        for b in range(bpc):
            ari[c, b] = _idx16(aidx[c * bpc + b])

    import ml_dtypes

    dlc = np.ascontiguousarray(
        dlocp.reshape(n_cores, bpc, nchunk, P).transpose(0, 1, 3, 2)
    ).astype(ml_dtypes.bfloat16)  # [cores, bpc, P(edge), nchunk]

    # x permuted to out_row order for the dst-side ar mini-pass
    x_perm = np.zeros((slots, IN), dtype=np.float32)
    x_perm[out_row] = np.asarray(x, dtype=np.float32)

    # weights: Wcat [256, 272] = [W.T | B_l | B_r]
    W = np.asarray(W, dtype=np.float32)
    attn_l = np.asarray(attn_l, dtype=np.float32).reshape(H, C)
    attn_r = np.asarray(attn_r, dtype=np.float32).reshape(H, C)
    A_l = np.zeros((HC, H), dtype=np.float32)
    A_r = np.zeros((HC, H), dtype=np.float32)
    for h in range(H):
        A_l[h * C : (h + 1) * C, h] = attn_l[h]
        A_r[h * C : (h + 1) * C, h] = attn_r[h]
    WT = np.ascontiguousarray(W.T)  # [in, hc]
    wcat = np.concatenate([WT, WT @ A_l, WT @ A_r], axis=1)  # [256, 272]
    wcat = np.ascontiguousarray(wcat.reshape(2, P, PSROW)).astype(ml_dtypes.bfloat16)
    wcat_r = np.ascontiguousarray(
        (WT @ A_r).reshape(2, P, H)).astype(ml_dtypes.bfloat16)

    # x tiles for phase A: [T, 2, 128(in-lane), 128(node)], natural node order
    T = slots // P
    x_slot = np.zeros((slots, IN), dtype=np.float32)
    x_slot[:N] = np.asarray(x, dtype=np.float32)
    xt = np.ascontiguousarray(
        x_slot.reshape(T, P, 2, P).transpose(0, 2, 3, 1)
    ).astype(ml_dtypes.bfloat16)
    xt2 = np.ascontiguousarray(
        x_perm.reshape(n_cores, bpc, P, 2, P).transpose(0, 1, 3, 4, 2)
    ).astype(ml_dtypes.bfloat16)  # [cores, bpc, 2, in-lane, slot]

    iota = np.tile(np.arange(P, dtype=np.float32), (P, 1)).astype(
        ml_dtypes.bfloat16
    )  # iota[e, d] = d

    meta = dict(
        N=N, n_cores=n_cores, bpc=bpc, nchunk=nchunk, nch=nch,
        cbase=[int(v) for v in cbase], nranges=nranges,
        T=T, slots=slots, out_row=out_row,
    )
    shared = dict(xt=xt, wcat=wcat, wcat_r=wcat_r, iota=iota)
    per_core = [
        dict(dlc=dlc[c], xt2=xt2[c], ari=ari[c],
             **{f"x{r}": xpi[r][c] for r in range(nranges)})
        for c in range(n_cores)
    ]
    return meta, shared, per_core


# ---------------------------------------------------------------- device IR
def _build_program(meta):
    import concourse.bacc as bacc
    import concourse.bass as bass
    import concourse.tile as tile
    from concourse import mybir

    _install_tile_patch()

    bpc, nchunk, T = meta["bpc"], meta["nchunk"], meta["T"]
    nch, cbase, nranges = meta["nch"], meta["cbase"], meta["nranges"]
    n_cores = meta["n_cores"]
    f32 = mybir.dt.float32
    bf16 = mybir.dt.bfloat16
    i16 = mybir.dt.int16
    i32 = mybir.dt.int32
    Alu = mybir.AluOpType
    Act = mybir.ActivationFunctionType

    nc = bacc.Bacc("TRN2", target_bir_lowering=False, debug=False,
                   num_devices=n_cores, num_swdge_queues=max(NQ, 1),
                   dynamic_dma_scratch_size=24576)
    xt_in = nc.dram_tensor("xt", [T, 2, P, P], bf16, kind="ExternalInput").ap()
    wcat_in = nc.dram_tensor("wcat", [2, P, PSROW], bf16, kind="ExternalInput").ap()
    iota_in = nc.dram_tensor("iota", [P, P], bf16, kind="ExternalInput").ap()
    dlc_in = nc.dram_tensor("dlc", [bpc, P, nchunk], bf16, kind="ExternalInput").ap()
    xt2_in = nc.dram_tensor("xt2", [bpc, 2, P, P], bf16, kind="ExternalInput").ap()
    wcr_in = nc.dram_tensor("wcat_r", [2, P, H], bf16, kind="ExternalInput").ap()
    xr_in = {}
    for r in range(nranges):
        xr_in[r] = nc.dram_tensor(f"x{r}", [bpc, P, nch[r] * 8], i16,
                                  kind="ExternalInput").ap()
    ari_in = nc.dram_tensor("ari", [bpc, P, nchunk * 8], i16,
                            kind="ExternalInput").ap()
    out_ex = nc.dram_tensor("out", [bpc * P, HC], f32, kind="ExternalOutput").ap()

    # phase A tile grouping (amortize DMA): largest power of two dividing T, <=8
    G8 = 8
    while T % G8:
        G8 //= 2

    qrr = [0]  # round-robin SWDGE queue counter

    with tile.TileContext(nc) as tc:
        with (
            tc.tile_pool(name="const", bufs=1) as cpool,
            tc.tile_pool(name="dram", bufs=1, space="DRAM") as dpool,
        ):
            rrows = [min(RNG, T * P - r * RNG) for r in range(nranges)]
            tables = [dpool.tile([rrows[r], ROW], bf16, name=f"table{r}")
                      for r in range(nranges)]
            tabAR = dpool.tile([bpc * P, P], bf16)
            wc0 = cpool.tile([P, PSROW], bf16, tag="wc0")
            wc1 = cpool.tile([P, PSROW], bf16, tag="wc1")
            nc.sync.dma_start(wc0[:], wcat_in[0])
            nc.sync.dma_start(wc1[:], wcat_in[1])
            iota_t = cpool.tile([P, P], bf16, tag="iota")
            nc.sync.dma_start(iota_t[:], iota_in[:])

            # ---- phase A': per-core [ar] table in out_row order (own nodes)
            wr0 = cpool.tile([P, H], bf16, tag="wr0")
            wr1 = cpool.tile([P, H], bf16, tag="wr1")
            nc.sync.dma_start(wr0[:], wcr_in[0])
            nc.sync.dma_start(wr1[:], wcr_in[1])
            with (
                tc.tile_pool(name="par", bufs=3) as par,
                tc.tile_pool(name="par_ps", bufs=2, space="PSUM") as parps,
            ):
                for tb in range(bpc):
                    ld2 = par.tile([P, 2, P], bf16, tag="ld2")
                    nc.sync.dma_start(ld2[:], xt2_in[tb].rearrange("h p n -> p h n"))
                    ps2 = parps.tile([P, H], f32)
                    nc.tensor.matmul(ps2[:], lhsT=ld2[:, 0, :], rhs=wr0[:],
                                     start=True, stop=False)
                    nc.tensor.matmul(ps2[:], lhsT=ld2[:, 1, :], rhs=wr1[:],
                                     start=False, stop=True)
                    art = par.tile([P, H], bf16, tag="art")
                    nc.vector.tensor_copy(art[:], ps2[:])
                    nc.scalar.dma_start(tabAR[tb * P : (tb + 1) * P, 0:H],
                                          art[:])

            # ---- phase A: projection table
            with (
                tc.tile_pool(name="pa", bufs=3) as pa,
                tc.tile_pool(name="pa_ps", bufs=4, space="PSUM") as paps,
            ):
                for g in range(T // G8):
                    tiles = slice(g * G8, (g + 1) * G8)
                    ld0 = pa.tile([P, G8, P], bf16, tag="ld0")
                    ld1 = pa.tile([P, G8, P], bf16, tag="ld1")
                    nc.sync.dma_start(
                        ld0[:], xt_in[tiles, 0].rearrange("u p n -> p u n"))
                    nc.sync.dma_start(
                        ld1[:], xt_in[tiles, 1].rearrange("u p n -> p u n"))
                    sbX = pa.tile([P, G8, PSROW], bf16, tag="sbX")
                    for u in range(G8):
                        ps = paps.tile([P, PSROW], f32)
                        nc.tensor.matmul(ps[:], lhsT=ld0[:, u, :],
                                         rhs=wc0[:], start=True, stop=False)
                        nc.tensor.matmul(ps[:], lhsT=ld1[:, u, :],
                                         rhs=wc1[:], start=False, stop=True)
                        if u % 2 == 0:
                            nc.vector.tensor_copy(sbX[:, u, :], ps[:])
                        else:
                            nc.scalar.activation(out=sbX[:, u, :],
                                                 in_=ps[:], func=Act.Copy)
                    row0 = g * G8 * P
                    r = row0 // RNG
                    dst = tables[r][row0 - r * RNG : row0 - r * RNG + G8 * P,
                                    :].rearrange("(u p) r -> p u r", p=P)
                    nc.scalar.dma_start(dst[:, :, 0:PSROW], sbX[:])

            # ---- phase B: per supergroup gathers + per block attention
            nsg = math.ceil(bpc / SG_B)
            with (
                tc.tile_pool(name="gat", bufs=2) as gp,
                tc.tile_pool(name="gat3", bufs=4) as gp3,
                tc.tile_pool(name="small", bufs=3) as sp,
                tc.tile_pool(name="mt", bufs=2) as mp,
                tc.tile_pool(name="ps", bufs=2, space="PSUM") as psp,
            ):
                def gather(src_ap, idx_dram, nb, kcols, n_chunks, rowe, tag):
                    pool = gp3 if tag in ("R0", "A") else gp
                    it = sp.tile([P, nb, kcols], i16, tag=f"{tag}i",
                                 name=f"{tag}i")
                    nc.sync.dma_start(it[:], idx_dram)
                    gt = pool.tile([P, n_chunks, rowe], bf16, tag=f"{tag}g",
                                   name=f"{tag}g")
                    itf = it[:].rearrange("p b k -> p (b k)")
                    for c0 in range(0, n_chunks, GMAX):
                        cn = min(GMAX, n_chunks - c0)
                        nc.gpsimd.dma_gather(gt[:, c0 : c0 + cn, :], src_ap,
                                             itf[:, c0 * 8 : (c0 + cn) * 8],
                                             cn * P, cn * P, rowe,
                                             queue_num=qrr[0] % NQ)
                        qrr[0] += 1
                    return gt

                for s in range(nsg):
                    b0 = s * SG_B
                    nb = min(SG_B, bpc - b0)
                    bsl = slice(b0, b0 + nb)
                    Gx = {}
                    for r in range(nranges):
                        Gx[r] = gather(
                            tables[r][:],
                            xr_in[r][bsl].rearrange("b p k -> p b k"),
                            nb, nch[r] * 8, nb * nch[r], ROW, f"R{r}")
                    Ats = gather(
                        tabAR[:], ari_in[bsl].rearrange("b p k -> p b k"),
                        nb, nchunk * 8, nb * nchunk, P, "A")

                    for bi in range(nb):
                        b = b0 + bi
                        dlct = sp.tile([P, nchunk], bf16, tag="dlct")
                        nc.sync.dma_start(dlct[:], dlc_in[b])
                        # one-hot S2[e, j, d] = (d == dloc[e, j])
                        S2 = sp.tile([P, nchunk, P], bf16, tag="S2")
                        nc.vector.tensor_tensor(
                            out=S2[:],
                            in0=iota_t[:].unsqueeze(1).to_broadcast(
                                [P, nchunk, P]),
                            in1=dlct[:].unsqueeze(2).to_broadcast(
                                [P, nchunk, P]),
                            op=Alu.is_equal)
                        # logits: al (ride-along) + ar (dst gather)
                        lg = sp.tile([P, nchunk, H], f32, tag="lg")
                        for r in range(nranges):
                            k = nch[r]
                            al = Gx[r][:, bi * k : (bi + 1) * k,
                                       HC : HC + H]
                            ar = Ats[:, bi * nchunk + cbase[r] :
                                     bi * nchunk + cbase[r + 1], 0:H]
                            nc.vector.tensor_tensor(
                                out=lg[:, cbase[r] : cbase[r + 1], :],
                                in0=al, in1=ar, op=Alu.add)
                        lr = sp.tile([P, nchunk, H], f32, tag="lr")
                        nc.scalar.activation(out=lr[:], in_=lg[:],
                                             func=Act.Copy, scale=0.2)
                        nc.vector.tensor_tensor(out=lr[:], in0=lg[:],
                                                in1=lr[:], op=Alu.max)
                        MT = mp.tile([P, nchunk, HC + H], bf16, tag="MT")
                        nc.scalar.activation(out=MT[:, :, HC : HC + H],
                                             in_=lr[:], func=Act.Exp)
                        # messages: alpha * xp (per range slice, per head)
                        for r in range(nranges):
                            k = nch[r]
                            xpg = Gx[r][:, bi * k : (bi + 1) * k, 0:HC]
                            csl = slice(cbase[r], cbase[r + 1])
                            a4 = MT[:, csl, HC : HC + H].unsqueeze(3) \
                                .to_broadcast([P, k, H, C])
                            nc.vector.tensor_tensor(
                                out=MT[:, csl, 0:HC].rearrange(
                                    "p k (h c) -> p k h c", c=C),
                                in0=xpg.rearrange(
                                    "p k (h c) -> p k h c", c=C),
                                in1=a4, op=Alu.mult)
                        # scatter-add via one-hot matmuls
                        U = psp.tile([P, HC + H], f32)
                        for j in range(nchunk):
                            nc.tensor.matmul(U[:], lhsT=S2[:, j, :],
                                             rhs=MT[:, j, :],
                                             start=(j == 0),
                                             stop=(j == nchunk - 1))
                        den = sp.tile([P, H], f32, tag="den")
                        nc.vector.tensor_scalar(den[:], U[:, HC : HC + H],
                                                1e-6, None, Alu.max)
                        rec = sp.tile([P, H], f32, tag="rec")
                        nc.vector.reciprocal(rec[:], den[:])
                        ob = sp.tile([P, HC], f32, tag="ob")
                        r3 = rec[:].unsqueeze(2).to_broadcast([P, H, C])
                        nc.vector.tensor_tensor(
                            out=ob[:].rearrange("p (h c) -> p h c", c=C),
                            in0=U[:, 0:HC].rearrange("p (h c) -> p h c", c=C),
                            in1=r3, op=Alu.mult)
                        nc.scalar.dma_start(out_ex[b * P : (b + 1) * P, :],
                                            ob[:])
    nc.compile()
    return nc


# ---------------------------------------------------------------- runner
def _run(inputs, trace=False, n_cores=8):
    from concourse.bass_utils import run_bass_kernel_spmd

    x = np.asarray(inputs["x"])
    edge_index = np.asarray(inputs["edge_index"])
    meta, shared, per_core = _preprocess(
        x, edge_index, inputs["W"], inputs["attn_l"], inputs["attn_r"], n_cores
    )
    nc = _build_program(meta)
    in_maps = [{**shared, **pc} for pc in per_core]
    res = run_bass_kernel_spmd(nc, in_maps, list(range(n_cores)), trace=trace)
    shards = np.concatenate([res.results[c]["out"] for c in range(n_cores)], axis=0)
    out = shards[meta["out_row"]]
    return np.ascontiguousarray(out.astype(np.float32)), res, meta


def kernel(**inputs) -> np.ndarray:
    out, _, _ = _run(inputs, trace=False)
    return out
